# revision 1
# baseline (speedup 1.0000x reference)
"""CIM signed-magnitude linear kernel v2 — pipelined, PE-transposed.

Math identity (exact): y = (x_q @ w_q.T) * scale_x * scale_w.T + bias with
x_q = round(x / (max|x|/127 + eps)) per token, w_q likewise per out-channel.

v2 structure (vs the phase-serial v1):
  * NO DMA transposes (they serialize against all DMA copies).  Quantized
    bf16 tiles are transposed on the PE (128x128 blocks vs a bf16 identity)
    and evicted PSUM->SBUF by ACT/DVE.
  * One-pass quantize: out_bf16 = (x * xinv) + 384.0 on ACT/DVE/Pool.  The
    single f32->bf16 output rounding lands on the integer grid (values in
    [257,511] where the bf16 ulp is exactly 1.0), giving round-half-even to
    the integer — identical to jnp.round.  The -384 is folded into the
    PSUM eviction (Copy with bias=-384).
  * PE warmup stream: dummy transposes keep the PE continuously busy from
    t~4us so the 3us DVFS ramp completes before real matmuls (2.4GHz =
    213ns per 512-col bf16 matmul instead of 427+).
  * Loads x0..x3 then w0..w3 (w3 split in halves): the post-last-load tail
    is only w3's chain + m=3's 8 matmuls + evict + store.
  * Engine balance: DVE = x-reduces + scales + out-mult; Pool = w-reduces +
    w-quants + broadcast + out-scale-bias; ACT = quantize-casts + all PSUM
    evictions.  PE = 64 transposes + 32 matmuls + 4 row transposes.

Sharding: 8 cores = 4 token-shards x 2 out-feature shards, no collectives
(unchanged from v1).
"""

import os

os.environ.setdefault("JAX_PLATFORMS", "cpu")

import numpy as np

B, S, IN_F, OUT_F = 2, 1024, 1024, 1024
T = B * S
M_SHARDS, N_SHARDS = 4, 2
TC = T // M_SHARDS             # 512 tokens per core
OC = OUT_F // N_SHARDS         # 512 out-features per core
NT = TC // 128                 # 4 token tiles
NO = OC // 128                 # 4 out-feature tiles
KB = IN_F // 128               # 8 contraction blocks
WROW = IN_F + 1                # weight row with bias appended

EPS = 1e-8
INV127 = 1.0 / 127.0
INV16129 = 1.0 / 16129.0
# fp16 has 10 mantissa bits: on [1024,2048) the ulp is exactly 1.0, so the
# single f32->fp16 output rounding of (q + 1536) lands on the integer grid
# (bf16's 7-bit mantissa gives ulp 2.0 there - unusable).
MAGIC = 1536.0
MAGIC_Q = 1536.0
N_WARMUP = 44

_CACHE = {}


def _build_nc():
    import concourse.bass as bass
    import concourse.mybir as mybir
    import concourse.tile as tile
    from concourse.masks import make_identity

    F32 = mybir.dt.float32
    BF16 = mybir.dt.float16  # quantized-value dtype (fp16: exact ints to 2048)
    ALU = mybir.AluOpType
    ACTF = mybir.ActivationFunctionType
    AX = mybir.AxisListType

    nc = bass.Bass("TRN2", target_bir_lowering=False, debug=False)

    x_d = nc.dram_tensor("x", [TC, IN_F], F32, kind="ExternalInput").ap()
    wb_d = nc.dram_tensor("wb", [OC, WROW], F32, kind="ExternalInput").ap()
    out_d = nc.dram_tensor("out", [OC, TC], F32, kind="ExternalOutput").ap()

    x3 = x_d.rearrange("(q p) i -> p q i", p=128)     # [128, NT, IN_F]
    w3 = wb_d.rearrange("(r p) i -> p r i", p=128)    # [128, NO, WROW]

    with tile.TileContext(nc) as tc:
        with (
            tc.tile_pool(name="raw", bufs=1) as raw,
            tc.tile_pool(name="qb", bufs=1) as qb,
            tc.tile_pool(name="small", bufs=1) as small,
            tc.tile_pool(name="ot", bufs=2) as otp,
            tc.tile_pool(name="ob", bufs=2) as obp,
            tc.tile_pool(name="mmps", bufs=4, space="PSUM") as mmps,
            tc.tile_pool(name="stps", bufs=4, space="PSUM") as stps,
        ):
            x_sb = raw.tile([128, NT, IN_F], F32, tag="x_sb")
            w_sb = raw.tile([128, NO, WROW], F32, tag="w_sb")
            xq = qb.tile([128, NT, IN_F], BF16, tag="xq")     # values +384
            wq = qb.tile([128, NO, IN_F], BF16, tag="wq")     # values +384
            xqT = qb.tile([128, KB, TC], BF16, tag="xqT")     # true ints
            wqT = qb.tile([128, KB, OC], BF16, tag="wqT")     # true ints
            bcx = qb.tile([128, TC], F32, tag="bcx")          # sx/16129 bcast
            row_sb = qb.tile([1, TC], F32, tag="row_sb")
            ones1 = qb.tile([1, 128], F32, tag="ones1")
            ident_bf = qb.tile([128, 128], BF16, tag="ident_bf")
            ident_f = qb.tile([128, 128], F32, tag="ident_f")

            xmax = small.tile([128, NT], F32, tag="xmax")
            xden = small.tile([128, NT], F32, tag="xden")
            xinv = small.tile([128, NT], F32, tag="xinv")
            wmax = small.tile([128, NO], F32, tag="wmax")
            wmaxh = small.tile([128, 2], F32, tag="wmaxh")    # w3 half-maxes
            wden = small.tile([128, NO], F32, tag="wden")
            winv = small.tile([128, NO], F32, tag="winv")
            bias2 = small.tile([128, NO], F32, tag="bias2")

            ps = [mmps.tile([128, TC], F32, tag="ps", name=f"ps{m}")
                  for m in range(NO)]
            st = [stps.tile([128, 512], BF16, tag="st", name=f"st{i}")
                  for i in range(4)]

            # ---- constants ----
            nc.gpsimd.memset(ones1, 1.0)
            make_identity(nc, ident_bf)
            make_identity(nc, ident_f)

            # ---- loads: x tiles first, w tiles after, w3 split ----
            for q in range(NT):
                nc.sync.dma_start(out=x_sb[:, q:q + 1], in_=x3[:, q:q + 1])
            for r in range(NO - 1):
                nc.sync.dma_start(out=w_sb[:, r:r + 1], in_=w3[:, r:r + 1])
            nc.sync.dma_start(out=w_sb[:, 3, 0:512], in_=w3[:, 3, 0:512])
            nc.sync.dma_start(out=w_sb[:, 3, 512:WROW], in_=w3[:, 3, 512:WROW])

            # ---- PE warmup: keep the clock ramping during loads ----
            for i in range(N_WARMUP):
                nc.tensor.transpose(st[0][:, 0:128], ident_bf, ident_bf)

            def x_red(q):
                nc.vector.tensor_reduce(
                    out=xmax[:, q:q + 1], in_=x_sb[:, q, :], axis=AX.X,
                    op=ALU.max, apply_absolute_value=True)
                nc.vector.tensor_scalar(
                    out=xden[:, q:q + 1], in0=xmax[:, q:q + 1],
                    scalar1=INV127, scalar2=EPS, op0=ALU.mult, op1=ALU.add)
                nc.vector.reciprocal(out=xinv[:, q:q + 1], in_=xden[:, q:q + 1])

            def x_quant_act(q, c0, c1):
                nc.scalar.activation(
                    out=xq[:, q, c0:c1], in_=x_sb[:, q, c0:c1], func=ACTF.Copy,
                    scale=xinv[:, q:q + 1], bias=MAGIC_Q)

            def x_quant_pool(q, c0, c1):
                nc.gpsimd.tensor_scalar(
                    out=xq[:, q, c0:c1], in0=x_sb[:, q, c0:c1],
                    scalar1=xinv[:, q:q + 1], scalar2=MAGIC_Q,
                    op0=ALU.mult, op1=ALU.add)

            def x_T(q):
                a, b = (0, 1) if q % 2 == 0 else (2, 3)
                for k in range(KB):
                    dst = st[a] if k < 4 else st[b]
                    nc.tensor.transpose(
                        dst[:, (k % 4) * 128:(k % 4 + 1) * 128],
                        xq[:, q, k * 128:(k + 1) * 128], ident_bf)
                return a, b

            def x_ev(q, half, bank, eng):
                # evict st[bank] -> xqT[:, half*4:(half+1)*4, q-block]
                out_ap = xqT[:, half * 4:(half + 1) * 4, q * 128:(q + 1) * 128]
                in_ap = bank.rearrange("p (k c) -> p k c", c=128)
                if eng == "act":
                    nc.scalar.activation(out=out_ap, in_=in_ap,
                                         func=ACTF.Copy, scale=1.0, bias=-MAGIC)
                else:
                    nc.vector.tensor_scalar(out=out_ap, in0=in_ap,
                                            scalar1=-MAGIC, scalar2=None,
                                            op0=ALU.add)

            def w_red(r, eng):
                e = nc.gpsimd if eng == "pool" else nc.vector
                e.tensor_reduce(
                    out=wmax[:, r:r + 1], in_=w_sb[:, r, 0:IN_F], axis=AX.X,
                    op=ALU.max, apply_absolute_value=True)

            def w_den(r):
                nc.vector.tensor_scalar(
                    out=wden[:, r:r + 1], in0=wmax[:, r:r + 1],
                    scalar1=INV127, scalar2=EPS, op0=ALU.mult, op1=ALU.add)
                nc.vector.reciprocal(out=winv[:, r:r + 1], in_=wden[:, r:r + 1])

            def w_quant(r, c0, c1, eng):
                if eng == "act":
                    nc.scalar.activation(
                        out=wq[:, r, c0:c1], in_=w_sb[:, r, c0:c1],
                        func=ACTF.Copy, scale=winv[:, r:r + 1], bias=MAGIC_Q)
                else:
                    e = nc.gpsimd if eng == "pool" else nc.vector
                    e.tensor_scalar(
                        out=wq[:, r, c0:c1], in0=w_sb[:, r, c0:c1],
                        scalar1=winv[:, r:r + 1], scalar2=MAGIC_Q,
                        op0=ALU.mult, op1=ALU.add)

            def w_T(r):
                a, b = (0, 1) if r % 2 == 0 else (2, 3)
                for k in range(KB):
                    dst = st[a] if k < 4 else st[b]
                    nc.tensor.transpose(
                        dst[:, (k % 4) * 128:(k % 4 + 1) * 128],
                        wq[:, r, k * 128:(k + 1) * 128], ident_bf)
                return a, b

            def w_ev(r, half, bank, eng):
                out_ap = wqT[:, half * 4:(half + 1) * 4, r * 128:(r + 1) * 128]
                in_ap = bank.rearrange("p (k c) -> p k c", c=128)
                if eng == "act":
                    nc.scalar.activation(out=out_ap, in_=in_ap,
                                         func=ACTF.Copy, scale=1.0, bias=-MAGIC)
                else:
                    nc.vector.tensor_scalar(out=out_ap, in0=in_ap,
                                            scalar1=-MAGIC, scalar2=None,
                                            op0=ALU.add)

            def mm_group(m):
                for k in range(KB):
                    nc.tensor.matmul(
                        ps[m], lhsT=wqT[:, k, m * 128:(m + 1) * 128],
                        rhs=xqT[:, k, :], start=(k == 0), stop=(k == KB - 1))

            def out_evict(m):
                otmp = otp.tile([128, TC], F32, tag="otmp", name=f"otmp{m}")
                nc.vector.tensor_tensor(out=otmp, in0=ps[m], in1=bcx,
                                        op=ALU.mult)
                osb = obp.tile([128, TC], F32, tag="osb", name=f"osb{m}")
                nc.gpsimd.tensor_scalar(
                    out=osb, in0=otmp, scalar1=wmax[:, m:m + 1],
                    scalar2=bias2[:, m:m + 1], op0=ALU.mult, op1=ALU.add)
                nc.sync.dma_start(out=out_d[m * 128:(m + 1) * 128, :], in_=osb)

            # ================= pipelined emission (approx time order) ======
            # --- x chains ---
            x_red(0)
            x_quant_act(0, 0, 512)
            x_quant_pool(0, 512, IN_F)
            x_T(0)
            x_red(1)
            x_ev(0, 0, st[0], "act")
            x_ev(0, 1, st[1], "act")
            x_quant_act(1, 0, 512)
            x_quant_pool(1, 512, IN_F)
            x_T(1)
            x_red(2)
            x_ev(1, 0, st[2], "act")
            x_ev(1, 1, st[3], "act")
            x_quant_act(2, 0, 512)
            x_quant_pool(2, 512, IN_F)
            x_T(2)
            x_red(3)
            x_ev(2, 0, st[0], "act")
            x_ev(2, 1, st[1], "act")
            # x3 quant split ACT || DVE for a short tail
            x_quant_act(3, 0, 512)
            x_quant_pool(3, 512, IN_F)
            x_T(3)
            # token-scale row: 4 single-column PE transposes into ps[3]
            for q in range(NT):
                nc.tensor.transpose(
                    ps[3][0:1, q * 128:(q + 1) * 128], xmax[:, q:q + 1], ident_f)
            # row evict (* 1/16129) then PE ones-broadcast into ps[2]
            nc.scalar.activation(
                out=row_sb, in_=ps[3][0:1, :], func=ACTF.Copy,
                scale=INV16129, bias=0.0)
            nc.tensor.matmul(ps[2], lhsT=ones1, rhs=row_sb,
                             start=True, stop=True)
            x_ev(3, 0, st[2], "act")
            x_ev(3, 1, st[3], "act")

            # --- w chains ---
            w_red(0, "dve")
            w_den(0)
            w_quant(0, 0, IN_F, "pool")
            nc.gpsimd.tensor_copy(out=bias2[:, 0:1], in_=w_sb[:, 0, IN_F:WROW])
            w_T(0)
            w_red(1, "dve")
            w_den(1)
            w_ev(0, 0, st[0], "act")
            w_ev(0, 1, st[1], "act")
            w_quant(1, 0, IN_F, "pool")
            nc.gpsimd.tensor_copy(out=bias2[:, 1:2], in_=w_sb[:, 1, IN_F:WROW])
            w_T(1)
            w_red(2, "dve")
            w_den(2)
            w_ev(1, 0, st[2], "act")
            w_ev(1, 1, st[3], "act")
            mm_group(0)
            w_quant(2, 0, IN_F, "pool")
            nc.gpsimd.tensor_copy(out=bias2[:, 2:3], in_=w_sb[:, 2, IN_F:WROW])
            w_T(2)
            # w3: reduce halves on DVE, den on DVE, quant Pool||ACT
            nc.vector.tensor_reduce(
                out=wmaxh[:, 0:1], in_=w_sb[:, 3, 0:512], axis=AX.X,
                op=ALU.max, apply_absolute_value=True)
            nc.vector.tensor_reduce(
                out=wmaxh[:, 1:2], in_=w_sb[:, 3, 512:IN_F], axis=AX.X,
                op=ALU.max, apply_absolute_value=True)
            nc.vector.tensor_tensor(
                out=wmax[:, 3:4], in0=wmaxh[:, 0:1], in1=wmaxh[:, 1:2],
                op=ALU.max)
            w_den(3)
            w_ev(2, 0, st[0], "act")
            w_ev(2, 1, st[1], "act")
            mm_group(1)
            w_quant(3, 0, 512, "pool")
            w_quant(3, 512, IN_F, "act")
            nc.gpsimd.tensor_copy(out=bias2[:, 3:4], in_=w_sb[:, 3, IN_F:WROW])
            w_T(3)
            w_ev(3, 0, st[2], "act")
            w_ev(3, 1, st[3], "act")
            nc.vector.tensor_copy(out=bcx, in_=ps[2])
            out_evict(0)
            mm_group(2)
            out_evict(1)
            mm_group(3)
            out_evict(2)
            out_evict(3)

    _split_multiwaits(nc)
    return nc


def _split_multiwaits(nc):
    """Hoist all but one wait of any multi-wait instruction into standalone
    EventSemaphore instructions (the ISA carries one wait per instruction)."""
    import concourse.mybir as mybir

    fn = nc.m.functions[0]
    ctr = [0]
    for blk in fn.blocks:
        insts = list(blk.instructions)
        changed = False
        out = []
        for inst in insts:
            si = inst.sync_info
            waits = list(si.on_wait or []) if si is not None else []
            if len(waits) > 1:
                for w in waits[:-1]:
                    ctr[0] += 1
                    es = mybir.InstEventSemaphore(
                        name=f"I-eswait-{ctr[0]}", engine=inst.engine,
                        ins=[], outs=[],
                    )
                    es.sync_info = mybir.SyncInfo(on_wait=[w], on_update=[])
                    out.append(es)
                    nc.register_instruction(es)
                inst.sync_info = mybir.SyncInfo(
                    on_wait=[waits[-1]], on_update=list(si.on_update or []),
                )
                changed = True
            out.append(inst)
        if changed:
            blk.instructions = out


def get_nc():
    if "nc" not in _CACHE:
        _CACHE["nc"] = _build_nc()
    return _CACHE["nc"]


def make_in_maps(x, weight, bias):
    xf = np.ascontiguousarray(np.asarray(x, dtype=np.float32).reshape(T, IN_F))
    w = np.asarray(weight, dtype=np.float32)
    b = np.asarray(bias, dtype=np.float32)
    wb = np.concatenate([w, b[:, None]], axis=1)
    in_maps = []
    for c in range(M_SHARDS * N_SHARDS):
        im, jn = divmod(c, N_SHARDS)
        in_maps.append({
            "x": np.ascontiguousarray(xf[im * TC:(im + 1) * TC]),
            "wb": np.ascontiguousarray(wb[jn * OC:(jn + 1) * OC]),
        })
    return in_maps


def assemble(results):
    y = np.empty((T, OUT_F), dtype=np.float32)
    for c in range(M_SHARDS * N_SHARDS):
        im, jn = divmod(c, N_SHARDS)
        y[im * TC:(im + 1) * TC, jn * OC:(jn + 1) * OC] = results[c]["out"].T
    return y.reshape(B, S, OUT_F)


def run(x, weight, bias, **spmd_kwargs):
    from concourse.bass_utils import run_bass_kernel_spmd

    nc = get_nc()
    in_maps = make_in_maps(x, weight, bias)
    res = run_bass_kernel_spmd(nc, in_maps, core_ids=list(range(8)), **spmd_kwargs)
    return assemble(res.results), res


def kernel(x, weight, bias):
    y, _ = run(x, weight, bias)
    return y

